# revision 14
# baseline (speedup 1.0000x reference)
"""Trainium2 Bass kernel for the Consis_Reg MSE loss.

Reference semantics (N=8192, D=512, C=64 classes):
    S[i,j]    = ||a_i - a_j||^2
    per_row_i = sum_{j: t_j == t_i} S[i,j] / cnt_{t_i}
    loss      = sum_i per_row_i

Class-aggregation identity (exact in real arithmetic):
    sum_{j in c} S[i,j] = cnt_c * ||a_i||^2 + sumSq_c - 2 a_i . sumA_c
    =>  loss = 2 * ( total_sumsq - sum_c ||sumA_c||^2 / cnt_c )
where, per class c:
    sumA_c  = sum_{i in c} a_i          (vector in R^D)
    cnt_c   = |{i : t_i == c}|
and total_sumsq = sum_i ||a_i||^2.

Each of the 8 cores processes a 1024-row shard of A:
    osum [64, 512] = M^T @ A_shard   (M = one-hot of targets; PSUM-accumulated
                                      float32r matmuls run at full PE speed and
                                      their tf32-like rounding only perturbs
                                      the small ||sumA_c||^2 correction term)
    ocnt [1, 64]   = per-class count (DVE reduce + GpSimd partition reduce)
    osq  [1, 1]    = sum of squares of the shard, computed in exact fp32 by
                     bitcasting the f32r bytes back to f32 on the DVE/GpSimd
The host sums the 8 partials and folds them into the final scalar.

Rows are assigned to SBUF partitions in contiguous blocks (partition p gets
rows p*8..p*8+7 of the shard) so input DMAs move 8KB-contiguous chunks per
partition; the matmul contraction is invariant to row order because the
one-hot rows are permuted identically.
"""

import numpy as np

N, D, C = 8192, 512, 64
NCORES = 8
ROWS = N // NCORES  # rows per core
P = 128             # SBUF partitions
NT = ROWS // P      # row-tiles per core (rows per partition)
NQ = 4              # input DMA / sumsq chunks
QT = NT // NQ       # row-tiles per chunk

_PROGRAM_CACHE = {}


def _build_program():
    import concourse.bass as bass
    import concourse.bacc as bacc
    import concourse.tile as tile
    from concourse import bass_isa, mybir

    f32 = mybir.dt.float32
    f32r = mybir.dt.float32r
    i32 = mybir.dt.int32

    nc = bacc.Bacc(
        "TRN2", target_bir_lowering=False, debug=False, num_devices=NCORES
    )
    a_dram = nc.dram_tensor("a", [P, NT, D], f32r, kind="ExternalInput").ap()
    t_dram = nc.dram_tensor("t", [P, NT], i32, kind="ExternalInput").ap()
    osum = nc.dram_tensor("osum", [C, D], f32, kind="ExternalOutput").ap()
    ocnt = nc.dram_tensor("ocnt", [1, C], f32, kind="ExternalOutput").ap()
    osq = nc.dram_tensor("osq", [1, 1], f32, kind="ExternalOutput").ap()

    with tile.TileContext(nc) as tc:
        with (
            tc.tile_pool(name="big", bufs=1) as big,
            tc.tile_pool(name="small", bufs=1) as small,
            tc.tile_pool(name="psum", bufs=1, space="PSUM") as pspool,
        ):
            # iota over [NT, C] free dims: value = class index c, directly
            # in f32 (exact for c < 64)
            iota_f = small.tile([P, NT, C], f32)
            nc.gpsimd.iota(
                iota_f,
                pattern=[[0, NT], [1, C]],
                base=0,
                channel_multiplier=0,
                allow_small_or_imprecise_dtypes=True,
            )

            t_sb = small.tile([P, NT], i32)
            nc.sync.dma_start(out=t_sb, in_=t_dram)
            t_f = small.tile([P, NT], f32)
            nc.vector.tensor_copy(t_f, t_sb)
            # broadcast t along the class dim: [P, NT, C] with stride 0 on C
            t_b = bass.AP(
                tensor=t_f.tensor,
                offset=t_f.offset,
                ap=[t_f.ap[0], t_f.ap[1], [0, C]],
            )

            a_sb = big.tile([P, NT, D], f32r)
            for q in range(NQ):
                lo, hi = q * QT, (q + 1) * QT
                nc.sync.dma_start(out=a_sb[:, lo:hi, :], in_=a_dram[:, lo:hi, :])

            # one-hot blocks M[p, r, c] = (t[p, r] == c) in a single DVE op
            m_all = big.tile([P, NT, C], f32r)
            nc.vector.tensor_tensor(
                m_all, iota_f, t_b, mybir.AluOpType.is_equal
            )

            # per-partition sum of squares in exact fp32 (square + row-sum in
            # one op; alternate DVE / ACT per chunk to balance engines)
            sq_scr = big.tile([P, QT * D], f32, tag="sq_scr")
            sq_scr2 = big.tile([P, QT * D], f32, tag="sq_scr2")
            sqp = small.tile([P, NQ], f32)
            for q in range(NQ):
                lo, hi = q * QT, (q + 1) * QT
                av = a_sb[:, lo:hi, :].bitcast(f32).rearrange("p a d -> p (a d)")
                if q % 2 == 0:
                    nc.vector.scalar_tensor_tensor(
                        out=sq_scr,
                        in0=av,
                        scalar=1.0,
                        in1=av,
                        op0=mybir.AluOpType.mult,
                        op1=mybir.AluOpType.mult,
                        accum_out=sqp[:, q : q + 1],
                    )
                else:
                    nc.scalar.activation(
                        sq_scr2,
                        av,
                        mybir.ActivationFunctionType.Square,
                        accum_out=sqp[:, q : q + 1],
                    )

            # PSUM-accumulated class sums: osum = sum_r M_r^T @ A_r
            psum_s = pspool.tile([C, D], f32)
            for r in range(NT):
                nc.tensor.matmul(
                    psum_s,
                    lhsT=m_all[:, r, :],
                    rhs=a_sb[:, r, :],
                    start=(r == 0),
                    stop=(r == NT - 1),
                )

            # counts: sum M over the NT axis (DVE), then over partitions
            cnt_sum = small.tile([P, C], f32)
            nc.vector.reduce_sum(
                cnt_sum,
                m_all.bitcast(f32).rearrange("p a c -> p c a"),
                axis=mybir.AxisListType.X,
            )
            cnt_red = small.tile([P, C], f32)
            nc.gpsimd.partition_all_reduce(
                cnt_red, cnt_sum, channels=P, reduce_op=bass_isa.ReduceOp.add
            )
            nc.sync.dma_start(out=ocnt, in_=cnt_red[0:1, :])

            # total sumsq: reduce the chunk partials, then over partitions
            sq1 = small.tile([P, 1], f32)
            nc.vector.reduce_sum(sq1, sqp, axis=mybir.AxisListType.X)
            sq_red = small.tile([P, 1], f32)
            nc.gpsimd.partition_all_reduce(
                sq_red, sq1, channels=P, reduce_op=bass_isa.ReduceOp.add
            )
            nc.sync.dma_start(out=osq, in_=sq_red[0:1, :])

            # class sums: PSUM -> SBUF -> DRAM
            osum_sb = small.tile([C, D], f32)
            nc.vector.tensor_copy(osum_sb, psum_s)
            nc.sync.dma_start(out=osum, in_=osum_sb)

    nc.compile()
    return nc


def get_program():
    if "nc" not in _PROGRAM_CACHE:
        _PROGRAM_CACHE["nc"] = _build_program()
    return _PROGRAM_CACHE["nc"]


def make_in_maps(representations, targets):
    A = np.ascontiguousarray(np.asarray(representations, dtype=np.float32))
    t = np.ascontiguousarray(np.asarray(targets).astype(np.int32))
    in_maps = []
    for core in range(NCORES):
        a_sh = A[core * ROWS : (core + 1) * ROWS].reshape(P, NT, D)
        t_sh = t[core * ROWS : (core + 1) * ROWS].reshape(P, NT)
        in_maps.append({"a": a_sh, "t": t_sh})
    return in_maps


def combine_partials(results):
    sums = np.zeros((C, D), np.float64)
    cnt = np.zeros(C, np.float64)
    total_sumsq = 0.0
    for r in results:
        sums += r["osum"].astype(np.float64)
        cnt += r["ocnt"].astype(np.float64)[0]
        total_sumsq += float(r["osq"][0, 0])
    loss = 2.0 * (total_sumsq - ((sums * sums).sum(axis=1) / cnt).sum())
    return np.float32(loss)


def kernel(representations, targets):
    from concourse.bass_utils import run_bass_kernel_spmd

    nc = get_program()
    in_maps = make_in_maps(representations, targets)
    res = run_bass_kernel_spmd(nc, in_maps, list(range(NCORES)))
    return combine_partials(res.results)


# revision 20
# speedup vs baseline: 1.1289x; 1.1289x over previous
"""Trainium2 Bass kernel for the Consis_Reg MSE loss.

Reference semantics (N=8192, D=512, C=64 classes):
    S[i,j]    = ||a_i - a_j||^2
    per_row_i = sum_{j: t_j == t_i} S[i,j] / cnt_{t_i}
    loss      = sum_i per_row_i

Class-aggregation identity (exact in real arithmetic):
    sum_{j in c} S[i,j] = cnt_c * ||a_i||^2 + sumSq_c - 2 a_i . sumA_c
    =>  loss = 2 * ( total_sumsq - sum_c ||sumA_c||^2 / cnt_c )
where, per class c:
    sumA_c  = sum_{i in c} a_i          (vector in R^D)
    cnt_c   = |{i : t_i == c}|
and total_sumsq = sum_i ||a_i||^2.

Each of the 8 cores processes a 1024-row shard of A:
    osum [64, 512] = M^T @ A_shard   (M = one-hot of targets; PSUM-accumulated
                                      float32r matmuls run at full PE speed and
                                      their tf32-like rounding only perturbs
                                      the small ||sumA_c||^2 correction term)
    ocnt [1, 64]   = per-class count (DVE reduce + GpSimd partition reduce)
    osq  [1, 1]    = sum of squares of the shard, computed in exact fp32 by
                     bitcasting the f32r bytes back to f32 on the DVE/GpSimd
The host sums the 8 partials and folds them into the final scalar.

Rows are assigned to SBUF partitions in contiguous blocks (partition p gets
rows p*8..p*8+7 of the shard) so input DMAs move 8KB-contiguous chunks per
partition; the matmul contraction is invariant to row order because the
one-hot rows are permuted identically.
"""

import numpy as np

N, D, C = 8192, 512, 64
NCORES = 8
ROWS = N // NCORES  # rows per core
P = 128             # SBUF partitions
NT = ROWS // P      # row-tiles per core (rows per partition)
NQ = 4              # input DMA / sumsq chunks
QT = NT // NQ       # row-tiles per chunk

_PROGRAM_CACHE = {}


def _build_program():
    import concourse.bass as bass
    import concourse.bacc as bacc
    import concourse.tile as tile
    from concourse import mybir

    f32 = mybir.dt.float32
    f32r = mybir.dt.float32r
    i32 = mybir.dt.int32

    nc = bacc.Bacc(
        "TRN2", target_bir_lowering=False, debug=False, num_devices=NCORES
    )
    a_dram = nc.dram_tensor("a", [P, NT, D], f32r, kind="ExternalInput").ap()
    t_dram = nc.dram_tensor("t", [P, NT], i32, kind="ExternalInput").ap()
    osum = nc.dram_tensor("osum", [C, D], f32, kind="ExternalOutput").ap()
    ocnt = nc.dram_tensor("ocnt", [P, C], f32, kind="ExternalOutput").ap()
    osq = nc.dram_tensor("osq", [P, NQ], f32, kind="ExternalOutput").ap()

    with tile.TileContext(nc) as tc:
        with (
            tc.tile_pool(name="big", bufs=1) as big,
            tc.tile_pool(name="small", bufs=1) as small,
            tc.tile_pool(name="psum", bufs=1, space="PSUM") as pspool,
        ):
            # iota over [NT, C] free dims: value = class index c, directly
            # in f32 (exact for c < 64)
            iota_f = small.tile([P, NT, C], f32)
            nc.gpsimd.iota(
                iota_f,
                pattern=[[0, NT], [1, C]],
                base=0,
                channel_multiplier=0,
                allow_small_or_imprecise_dtypes=True,
            )

            t_sb = small.tile([P, NT], i32)
            nc.sync.dma_start(out=t_sb, in_=t_dram)
            t_f = small.tile([P, NT], f32)
            nc.vector.tensor_copy(t_f, t_sb)
            # broadcast t along the class dim: [P, NT, C] with stride 0 on C
            t_b = bass.AP(
                tensor=t_f.tensor,
                offset=t_f.offset,
                ap=[t_f.ap[0], t_f.ap[1], [0, C]],
            )

            # one tile per DMA chunk so downstream ops start as soon as
            # their chunk lands (Tile deps are whole-tile granular)
            a_q = []
            for q in range(NQ):
                lo, hi = q * QT, (q + 1) * QT
                aq = big.tile([P, QT, D], f32r, tag=f"a_q{q}")
                nc.sync.dma_start(out=aq, in_=a_dram[:, lo:hi, :])
                a_q.append(aq)

            # one-hot blocks M[p, r, c] = (t[p, r] == c) in a single DVE op
            m_all = big.tile([P, NT, C], f32r)
            nc.vector.tensor_tensor(
                m_all, iota_f, t_b, mybir.AluOpType.is_equal
            )

            # per-partition sum of squares in exact fp32 (square + row-sum in
            # one op; alternate DVE / ACT per chunk to balance engines)
            sq_scr = big.tile([P, QT * D], f32, tag="sq_scr")
            sq_scr2 = big.tile([P, QT * D], f32, tag="sq_scr2")
            sqp = small.tile([P, NQ], f32)
            for q in range(NQ):
                av = a_q[q].bitcast(f32).rearrange("p a d -> p (a d)")
                if q % 2 == 0:
                    nc.vector.scalar_tensor_tensor(
                        out=sq_scr,
                        in0=av,
                        scalar=1.0,
                        in1=av,
                        op0=mybir.AluOpType.mult,
                        op1=mybir.AluOpType.mult,
                        accum_out=sqp[:, q : q + 1],
                    )
                else:
                    nc.scalar.activation(
                        sq_scr2,
                        av,
                        mybir.ActivationFunctionType.Square,
                        accum_out=sqp[:, q : q + 1],
                    )

            # PSUM-accumulated class sums: osum = sum_r M_r^T @ A_r
            psum_s = pspool.tile([C, D], f32)
            for r in range(NT):
                nc.tensor.matmul(
                    psum_s,
                    lhsT=m_all[:, r, :],
                    rhs=a_q[r // QT][:, r % QT, :],
                    start=(r == 0),
                    stop=(r == NT - 1),
                )

            # counts: sum M over the NT axis (DVE); partition sum on host
            cnt_sum = small.tile([P, C], f32)
            nc.vector.reduce_sum(
                cnt_sum,
                m_all.bitcast(f32).rearrange("p a c -> p c a"),
                axis=mybir.AxisListType.X,
            )
            nc.sync.dma_start(out=ocnt, in_=cnt_sum)

            # sumsq chunk partials straight out; partition sum on host
            nc.sync.dma_start(out=osq, in_=sqp)

            # class sums: PSUM -> SBUF -> DRAM
            osum_sb = small.tile([C, D], f32)
            nc.vector.tensor_copy(osum_sb, psum_s)
            nc.sync.dma_start(out=osum, in_=osum_sb)

    nc.compile()
    return nc


def get_program():
    if "nc" not in _PROGRAM_CACHE:
        _PROGRAM_CACHE["nc"] = _build_program()
    return _PROGRAM_CACHE["nc"]


def make_in_maps(representations, targets):
    A = np.ascontiguousarray(np.asarray(representations, dtype=np.float32))
    t = np.ascontiguousarray(np.asarray(targets).astype(np.int32))
    in_maps = []
    for core in range(NCORES):
        a_sh = A[core * ROWS : (core + 1) * ROWS].reshape(P, NT, D)
        t_sh = t[core * ROWS : (core + 1) * ROWS].reshape(P, NT)
        in_maps.append({"a": a_sh, "t": t_sh})
    return in_maps


def combine_partials(results):
    sums = np.zeros((C, D), np.float64)
    cnt = np.zeros(C, np.float64)
    total_sumsq = 0.0
    for r in results:
        sums += r["osum"].astype(np.float64)
        cnt += r["ocnt"].astype(np.float64).sum(axis=0)
        total_sumsq += float(r["osq"].astype(np.float64).sum())
    loss = 2.0 * (total_sumsq - ((sums * sums).sum(axis=1) / cnt).sum())
    return np.float32(loss)


def kernel(representations, targets):
    from concourse.bass_utils import run_bass_kernel_spmd

    nc = get_program()
    in_maps = make_in_maps(representations, targets)
    res = run_bass_kernel_spmd(nc, in_maps, list(range(NCORES)))
    return combine_partials(res.results)
